# revision 18
# baseline (speedup 1.0000x reference)
"""L2-distance self-attention (B=2, N=2048, D=1024, H=16) on 8 trn2 NeuronCores, v3.

Sharding: core c handles batch c//4 and heads 4*(c%4) .. 4*(c%4)+4.

Key HW facts driving the design (measured on this device):
  - PE matmuls stream 1 col/cycle at 2.4 GHz only when K > 64; K<=64 runs at
    half rate.  So all big contractions are padded to K=128 (dead rows are
    free - cost is F-column driven).
  - Matmul PSUM dst must stay within one 2KB bank -> all matmuls emit F=512.
  - fp8 DoubleRow (K=2x128) doubles matmul throughput; used for PV.
  - ACT runs ~0.9ns/col; DVE tensor_scalar ~0.42ns/col (fp16 2x mode).

Engine split:
  PE : fp16 projections (K=128 tiles), fp16 S^T (K=128-padded aug tiles),
       fp8e4 DoubleRow PV, 1/den broadcast matmul, fp16 out-proj.
  ACT: sqrt (u = A*s via scale=A^2), half the proj->aug copies (Identity with
       per-partition bias column), softmax reciprocal (ACT table, from PSUM).
  DVE: Schraudolph exp: bits16 = sat_round(B8 - u/128); LOW byte is exactly
       float8_e4m3 of exp(-s)*2^(7*log2e) (scale cancels in softmax); PV reads
       the bits buffer via a stride-2 fp8e4 view.  Plus copies + normalize.
"""

import sys

for p in ("/opt/trn_rl_repo", "/root/.axon_site/_ro/trn_rl_repo"):
    if p not in sys.path:
        sys.path.append(p)

import numpy as np

B, N, D, H = 2, 2048, 1024, 16
HD = 64          # head dim
HPC = 4          # heads per core
HS = HPC * HD    # head-group width per core (256)
NB = N // 128    # 16 j blocks
KB = D // 128    # 8 contraction blocks for projections

A_CONST = 1024.0 * np.log2(np.e)          # 1477.3197
A2_CONST = float(A_CONST * A_CONST)
SCHR_SCALE = -1.0 / 128.0
SCHR_B = float(8 * np.log2(np.e) * 7.0 + 56.0 - 8.0 * 0.043)   # 136.447

_CACHE = {}


def _enable_ldw_opt():
    """Let walrus dedupe back-to-back same-weight LDWEIGHTS (S^T emits 4
    matmuls per j-block sharing one lhsT).  bass hardcodes the flag off."""
    import concourse.bass_utils as _bu

    if getattr(_bu, "_ldw_opt_patched", False):
        return
    _orig = _bu.run_command

    def _patched(argv, **kwargs):
        argv = [
            "--enable-ldw-opt=true" if a == "--enable-ldw-opt=false" else a
            for a in argv
        ]
        return _orig(argv, **kwargs)

    _bu.run_command = _patched
    _bu._ldw_opt_patched = True


def _build():
    import concourse.bacc as bacc
    import concourse.mybir as mybir
    import concourse.tile as tile

    dt = mybir.dt
    AF = mybir.ActivationFunctionType
    ALU = mybir.AluOpType
    PM = mybir.MatmulPerfMode

    nc = bacc.Bacc("TRN2", target_bir_lowering=False, debug=False)

    # ---- DRAM I/O (per core) ----
    xT = nc.dram_tensor("xT", [D, N], dt.float16, kind="ExternalInput")
    wq = nc.dram_tensor("wqT", [D, HS], dt.float16, kind="ExternalInput")
    wk = nc.dram_tensor("wkT", [D, HS], dt.float16, kind="ExternalInput")
    wv = nc.dram_tensor("wv_aug", [D + 1, HS], dt.float16, kind="ExternalInput")
    wo = nc.dram_tensor("woT", [HS, D], dt.float16, kind="ExternalInput")
    bcols = nc.dram_tensor("bcols", [128, 4], dt.float32, kind="ExternalInput")
    y = nc.dram_tensor("y", [N, D], dt.float32, kind="ExternalOutput")

    def act_recip(out, in_):
        """ACT-table reciprocal (bass blocks the wrapper; emit directly)."""
        se = nc.scalar
        inputs = [se.lower_ap(in_)]
        for arg in (0.0, 1.0, 0.0):   # bias, scale, alpha
            inputs.append(mybir.ImmediateValue(dtype=mybir.dt.float32, value=arg))
        return se.add_instruction(
            mybir.InstActivation(
                name=se.bass.get_next_instruction_name(),
                func=AF.Reciprocal,
                ins=inputs,
                outs=[se.lower_ap(out)],
            )
        )

    with tile.TileContext(nc) as tc:
        with (
            tc.tile_pool(name="cst", bufs=1) as cst,
            tc.tile_pool(name="u4", bufs=2) as u4,
            tc.tile_pool(name="up", bufs=6) as up,
            tc.tile_pool(name="wp", bufs=1) as wp,
            tc.tile_pool(name="wop", bufs=1) as wop,
            tc.tile_pool(name="aug", bufs=1) as aug,
            tc.tile_pool(name="recp", bufs=4) as recp,
            tc.tile_pool(name="pss", bufs=2, space="PSUM") as pss,   # 2-bank slots
            tc.tile_pool(name="psv", bufs=1, space="PSUM") as psv,   # 2x 2-bank halves
        ):
            # ---- constants ----
            ones_row = cst.tile([1, N], dt.float16, tag="ones_row")
            nc.gpsimd.memset(ones_row[:], 1.0)
            # K=128-padded colsum producers: rows 0..63 active, 64..127 zero.
            # e66q col64 = 1 (q2 row); e66k col65 = 0.25 (k2 = 0.25*sum(kb2^2)).
            e66q = cst.tile([128, 66], dt.float16, tag="e66q")
            nc.gpsimd.memset(e66q[:], 0.0)
            nc.gpsimd.memset(e66q[0:64, 64:65], 1.0)
            e66k = cst.tile([128, 66], dt.float16, tag="e66k")
            nc.gpsimd.memset(e66k[:], 0.0)
            nc.gpsimd.memset(e66k[0:64, 65:66], 0.25)
            bcol = cst.tile([128, 4], dt.float32, tag="bcol")
            nc.sync.dma_start(bcol[:], bcols[:, :])
            ones_f32 = cst.tile([1, 2], dt.float32, tag="ones_f32")
            nc.gpsimd.memset(ones_f32[:], 1.0)
            # aug-row add-constants: col 0 = 1@row65 (q: ones row), col 1 =
            # 1@row64 (k: ones row); zero elsewhere.  Partition-65 writes are
            # only legal via DMA.
            rowc = cst.tile([128, 2], dt.float32, tag="rowc")
            nc.gpsimd.memset(rowc[:], 0.0)
            nc.sync.dma_start(rowc[65:66, 0:1], ones_f32[0:1, 0:1])
            nc.sync.dma_start(rowc[64:65, 1:2], ones_f32[0:1, 1:2])

            # PE warmup
            for w in range(2):
                wup = pss.tile([128, 512], dt.float32, tag="b2", name="wup")
                for r in range(12):
                    nc.tensor.matmul(
                        wup[:], ones_row[0:1, 0:128], ones_row[0:1, 0:512],
                        start=(r == 0), stop=(r == 11),
                    )

            # ---- load inputs ----
            xt = [u4.tile([128, N], dt.float16, tag="xt", name=f"xt{k}", bufs=8)
                  for k in range(KB)]
            wq_all = wp.tile([128, KB * HS], dt.float16, tag="wq_all")
            wk_all = wp.tile([128, KB * HS], dt.float16, tag="wk_all")
            wv_all = wp.tile([128, KB * HS], dt.float16, tag="wv_all")
            brows = wp.tile([1, HS], dt.float16, tag="brows")
            for k in range(KB):
                eng = nc.sync if k % 2 == 0 else nc.gpsimd
                eng.dma_start(xt[k][:], xT[k * 128 : (k + 1) * 128, :])
                nc.sync.dma_start(
                    wq_all[:, k * HS : (k + 1) * HS], wq[k * 128 : (k + 1) * 128, :]
                )
            for k in range(KB):
                nc.sync.dma_start(
                    wk_all[:, k * HS : (k + 1) * HS], wk[k * 128 : (k + 1) * 128, :]
                )
            nc.gpsimd.dma_start(brows[:, :], wv[D : D + 1, :])
            for k in range(KB):
                nc.gpsimd.dma_start(
                    wv_all[:, k * HS : (k + 1) * HS], wv[k * 128 : (k + 1) * 128, :]
                )

            # ---- big buffers (aug padded to K=128; rows 66..127 zero) ----
            q_aug = [aug.tile([128, N], dt.float16, tag=f"qa{h}", name=f"qa{h}")
                     for h in range(HPC)]
            k_stat = [aug.tile([128, N], dt.float16, tag=f"ks{h}", name=f"ks{h}")
                      for h in range(HPC)]
            for h in range(HPC):
                nc.gpsimd.memset(q_aug[h][:], 0.0)
                nc.gpsimd.memset(k_stat[h][:], 0.0)
            # v8: [128 j, (4 h, 16 jb, 128 m)] e4m3; m = [v(64) | ones | pad0]
            v8 = aug.tile([128, HPC * NB * 128], dt.float8e4, tag="v8")
            v8r = v8[:].rearrange("p (h t m) -> p h t m", h=HPC, m=128)
            nc.gpsimd.memset(v8[:], 0.0)
            nc.gpsimd.memset(v8r[:, :, :, 64:65], 1.0)
            # bits ring: 4 slots x [2 jb, 2048 i] u16
            bits = aug.tile([128, 4 * 2 * N], dt.uint16, tag="bits")
            bits_e4 = bits[:].bitcast(dt.float8e4)
            bits_r = bits[:].rearrange("p (s t i) -> p s t i", s=4, t=2)
            oTp = [
                aug.tile([128, N], dt.float16, tag="oTp0", name="oTp0"),
                aug.tile([128, N], dt.float16, tag="oTp1", name="oTp1"),
            ]
            oTraw = [
                aug.tile([128, N], dt.float16, tag="oTr0", name="oTr0"),
                aug.tile([128, N], dt.float16, tag="oTr1", name="oTr1"),
            ]
            den_sb = [aug.tile([1, N], dt.float16, tag=f"den{h}", name=f"den{h}")
                      for h in range(HPC)]

            # ---- q/k projections; bias added at the psum->aug copy ----
            def proj_qk_c(w_all, bc_idx, dest, m, c):
                p = pss.tile([128, 1024], dt.float32, tag="b2", name="pp")
                lo = c * 1024
                for sc in range(2):
                    fo = sc * 512
                    for k in range(KB):
                        nc.tensor.matmul(
                            p[:, fo : fo + 512],
                            w_all[:, k * HS + m * 128 : k * HS + (m + 1) * 128],
                            xt[k][:, lo + fo : lo + fo + 512],
                            start=(k == 0), stop=(k == KB - 1),
                        )
                # psum -> aug rows with per-partition bias column (DVE)
                nc.vector.tensor_scalar(
                    out=dest[2 * m][0:64, lo : lo + 1024],
                    in0=p[0:64, :],
                    scalar1=bcol[0:64, bc_idx : bc_idx + 1], scalar2=None,
                    op0=ALU.add,
                )
                nc.vector.tensor_scalar(
                    out=dest[2 * m + 1][0:64, lo : lo + 1024],
                    in0=p[64:128, :],
                    scalar1=bcol[64:128, bc_idx : bc_idx + 1], scalar2=None,
                    op0=ALU.add,
                )

            def norms_part(h, which):
                src_tile = q_aug[h] if which == "q" else k_stat[h]
                emat = e66q if which == "q" else e66k
                rc = 0 if which == "q" else 1
                sq = u4.tile([128, N], dt.float16, tag="sq", name="sq", bufs=2)
                nc.vector.tensor_tensor(
                    out=sq[:], in0=src_tile[:], in1=src_tile[:], op=ALU.mult,
                )
                for c in range(2):
                    lo = c * 1024
                    p = pss.tile([66, 1024], dt.float32, tag="b2", name="np")
                    for sc in range(2):
                        fo = sc * 512
                        nc.tensor.matmul(
                            p[:, fo : fo + 512], emat[:],
                            sq[:, lo + fo : lo + fo + 512],
                            start=True, stop=True,
                        )
                    # rows 64:66 = (q2, 0)+(0, 1) for q; (0, k2)+(1, 0) for k
                    nc.vector.tensor_scalar(
                        out=src_tile[64:66, lo : lo + 1024],
                        in0=p[64:66, :],
                        scalar1=rowc[64:66, rc : rc + 1], scalar2=None,
                        op0=ALU.add,
                    )

            # ---- v projection (bias added at the fp8 copy via bv bcast) ----
            bvb = wp.tile([128, HS], dt.float16, tag="bvb")

            def bvb_init():
                p = pss.tile([128, HS], dt.float32, tag="b2", name="bvbp")
                nc.tensor.matmul(p[:], ones_row[0:1, 0:128], brows[0:1, :],
                                 start=True, stop=True)
                nc.scalar.copy(bvb[:], p[:])

            def vp_one(jb):
                p = pss.tile([128, HS], dt.float32, tag="b2", name="vp")
                for k in range(KB):
                    nc.tensor.matmul(
                        p[:], xt[k][:, jb * 128 : (jb + 1) * 128],
                        wv_all[:, k * HS : (k + 1) * HS],
                        start=(k == 0), stop=(k == KB - 1),
                    )
                with nc.allow_low_precision(reason="fp8 attention values"):
                    nc.vector.tensor_tensor(
                        out=v8r[:, :, jb, 0:64],
                        in0=p[:].rearrange("p (h d) -> p h d", d=64),
                        in1=bvb[:].rearrange("p (h d) -> p h d", d=64),
                        op=ALU.add,
                    )

            # ---- S^T + sqrt + Schraudolph-exp ----
            def st_chunk(h, jb, c):
                lo = c * 1024
                stp = pss.tile([128, 1024], dt.float32, tag="b2", name="st")
                for sc in range(2):
                    fo = sc * 512
                    nc.tensor.matmul(
                        stp[:, fo : fo + 512],
                        k_stat[h][:, jb * 128 : (jb + 1) * 128],
                        q_aug[h][:, lo + fo : lo + fo + 512],
                        start=True, stop=True,
                    )
                u = up.tile([128, 1024], dt.float16, tag="u", name="u")
                nc.scalar.activation(u[:], stp[:], AF.Sqrt, scale=A2_CONST)
                slot = (jb // 2) % 4
                with nc.allow_low_precision(reason="fp8 softmax weights"):
                    nc.vector.tensor_scalar(
                        out=bits_r[:, slot, jb % 2, lo : lo + 1024],
                        in0=u[:], scalar1=SCHR_SCALE, scalar2=SCHR_B,
                        op0=ALU.mult, op1=ALU.add,
                    )

            def pv_pair(h, t, pv):
                slot = t % 4
                br = bits_e4.rearrange("p (s t i) -> p s t i", s=4, t=2)
                for sc in range(4):
                    bo_ = 2 * 512 * sc
                    half = pv[sc // 2]
                    nc.tensor.matmul(
                        half[:, 512 * (sc % 2) : 512 * (sc % 2) + 512],
                        v8r[:, h, 2 * t : 2 * t + 2, :],
                        br[:, slot, :, bo_ : bo_ + 1024 : 2],
                        start=(t == 0), stop=(t == NB // 2 - 1),
                        perf_mode=PM.DoubleRow,
                    )

            def normalize(h, pv):
                # Per-head: evacuate pv halves to SBUF only (reciprocal is
                # deferred to the tail: one ACT table switch total).  Each
                # 2-bank half releases independently so the next head's PV
                # can start while the other half is still draining.
                half = 64 * (h % 2)
                for c in range(2):
                    lo = c * 1024
                    with nc.allow_low_precision(reason="softmax denominator"):
                        nc.vector.tensor_copy(
                            den_sb[h][:, lo : lo + 1024], pv[c][64:65, :]
                        )
                        if h % 2 == 0:
                            nc.scalar.copy(
                                oTraw[h // 2][0:64, lo : lo + 1024], pv[c][0:64, :]
                            )
                        else:
                            nc.vector.tensor_copy(
                                oTraw[h // 2][64:128, lo : lo + 1024], pv[c][0:64, :]
                            )

            def normalize_tail(h):
                rec = recp.tile([1, N], dt.float16, tag="rec", name="rec")
                with nc.allow_low_precision(reason="softmax reciprocal"):
                    act_recip(rec[:], den_sb[h][:])
                half = 64 * (h % 2)
                for c in range(2):
                    lo = c * 1024
                    bc = pss.tile([128, 1024], dt.float32, tag="b2", name="bc")
                    for sc in range(2):
                        fo = sc * 512
                        nc.tensor.matmul(
                            bc[half : half + 64, fo : fo + 512],
                            ones_row[0:1, 0:64],
                            rec[0:1, lo + fo : lo + fo + 512],
                            start=True, stop=True,
                        )
                    bcs = u4.tile([128, 1024], dt.float16, tag="bcs", name="bcs",
                                  bufs=2)
                    nc.vector.tensor_copy(
                        bcs[half : half + 64, :], bc[half : half + 64, :]
                    )
                    with nc.allow_low_precision(reason="fp16 attention out"):
                        nc.vector.tensor_tensor(
                            out=oTp[h // 2][half : half + 64, lo : lo + 1024],
                            in0=oTraw[h // 2][half : half + 64, lo : lo + 1024],
                            in1=bcs[half : half + 64, :], op=ALU.mult,
                        )

            # ================= emission schedule =================
            proj_qk_c(wq_all, 0, q_aug, 0, 0)
            proj_qk_c(wk_all, 2, k_stat, 0, 0)
            proj_qk_c(wq_all, 0, q_aug, 0, 1)
            proj_qk_c(wk_all, 2, k_stat, 0, 1)
            norms_part(0, "q")
            norms_part(0, "k")
            norms_part(1, "q")
            norms_part(1, "k")

            wotp = [wop.tile([128, D], dt.float16, tag=f"wop{p}", name=f"wop{p}")
                    for p in range(2)]
            for p in range(2):
                nc.gpsimd.dma_start(wotp[p][:], wo[p * 128 : (p + 1) * 128, :])

            pv_tiles = {}

            def head_loop(h, weave):
                """weave[i] runs after jb i's st chunks, before the pair PV."""
                pv = [
                    psv.tile([128, 1024], dt.float32, tag=f"pv{c}", name=f"pv{h}_{c}")
                    for c in range(2)
                ]
                pv_tiles[h] = pv
                for jb in range(NB):
                    st_chunk(h, jb, 0)
                    st_chunk(h, jb, 1)
                    w = weave.get(jb)
                    if w:
                        w()
                    if jb % 2 == 1:
                        pv_pair(h, jb // 2, pv)

            bvb_init()
            vp_one(0)
            vp_one(1)
            weave0 = {}
            for jb in range(14):
                weave0[jb] = (lambda j=jb: vp_one(j + 2))
            weave0[6] = lambda: (vp_one(8), proj_qk_c(wq_all, 1, q_aug, 1, 0))
            weave0[7] = lambda: (vp_one(9), proj_qk_c(wq_all, 1, q_aug, 1, 1))
            weave0[8] = lambda: (vp_one(10), proj_qk_c(wk_all, 3, k_stat, 1, 0))
            weave0[9] = lambda: (vp_one(11), proj_qk_c(wk_all, 3, k_stat, 1, 1))
            weave0[10] = lambda: (vp_one(12), norms_part(2, "q"))
            weave0[11] = lambda: (vp_one(13), norms_part(2, "k"))
            weave0[12] = lambda: (vp_one(14), norms_part(3, "q"))
            weave0[13] = lambda: (vp_one(15), norms_part(3, "k"))
            head_loop(0, weave0)

            head_loop(1, {0: lambda: normalize(0, pv_tiles[0])})
            head_loop(2, {0: lambda: normalize(1, pv_tiles[1])})
            head_loop(3, {0: lambda: normalize(2, pv_tiles[2])})
            normalize(3, pv_tiles[3])
            for h in range(HPC):
                normalize_tail(h)

            # ---- output projection ----
            for ib in range(NB):
                yp = pss.tile([128, D], dt.float32, tag="b2", name="yp")
                for sc in range(2):
                    fo = sc * 512
                    for pr in range(2):
                        nc.tensor.matmul(
                            yp[:, fo : fo + 512],
                            oTp[pr][:, ib * 128 : (ib + 1) * 128],
                            wotp[pr][:, fo : fo + 512],
                            start=(pr == 0), stop=(pr == 1),
                        )
                yac = u4.tile([128, D], dt.float32, tag="yac", name="yac", bufs=2)
                if ib % 2 == 0:
                    nc.scalar.copy(yac[:], yp[:])
                    nc.sync.dma_start(y[ib * 128 : (ib + 1) * 128, :], yac[:])
                else:
                    nc.vector.tensor_copy(yac[:], yp[:])
                    nc.gpsimd.dma_start(y[ib * 128 : (ib + 1) * 128, :], yac[:])

    nc.compile()
    return nc


def _prep_in_maps(x, wq, bq, wk, bk, wv, bv, wo):
    f16 = np.float16
    in_maps = []
    xTs = [np.ascontiguousarray(x[b].T).astype(f16) for b in range(B)]
    for c in range(8):
        b, hg = divmod(c, HPC)
        hs = hg * HS
        wqT = np.ascontiguousarray(wq[hs : hs + HS, :].T).astype(f16)
        wkT = np.ascontiguousarray(-2.0 * wk[hs : hs + HS, :].T).astype(f16)
        wv_aug = np.concatenate(
            [wv[hs : hs + HS, :].T, bv[None, hs : hs + HS]], axis=0
        ).astype(f16)
        woT = np.ascontiguousarray(wo[:, hs : hs + HS].T).astype(f16)
        # bias columns [128, 4]: rows 0:64 even head, 64:128 odd head of the
        # m-pair; cols = (q m0, q m1, k m0, k m1); k pre-scaled by -2.
        bc = np.zeros((128, 4), np.float32)
        bc[:, 0] = bq[hs : hs + 128]
        bc[:, 1] = bq[hs + 128 : hs + 256]
        bc[:, 2] = -2.0 * bk[hs : hs + 128]
        bc[:, 3] = -2.0 * bk[hs + 128 : hs + 256]
        in_maps.append(
            {
                "xT": xTs[b],
                "wqT": wqT,
                "wkT": wkT,
                "wv_aug": np.ascontiguousarray(wv_aug),
                "woT": woT,
                "bcols": bc,
            }
        )
    return in_maps


def _get_nc():
    if "nc" not in _CACHE:
        _CACHE["nc"] = _build()
    return _CACHE["nc"]


def run(inputs, trace=False, **trace_kwargs):
    """Run on 8 cores; returns (full_output, BassKernelResults)."""
    from concourse.bass_utils import run_bass_kernel_spmd

    nc = _get_nc()
    in_maps = _prep_in_maps(
        np.asarray(inputs["x"], np.float32),
        np.asarray(inputs["wq"], np.float32), np.asarray(inputs["bq"], np.float32),
        np.asarray(inputs["wk"], np.float32), np.asarray(inputs["bk"], np.float32),
        np.asarray(inputs["wv"], np.float32), np.asarray(inputs["bv"], np.float32),
        np.asarray(inputs["wo"], np.float32),
    )
    res = run_bass_kernel_spmd(nc, in_maps, list(range(8)), trace=trace, **trace_kwargs)
    bo = np.asarray(inputs["bo"], np.float32)
    out = np.empty((B, N, D), np.float32)
    for b in range(B):
        acc = res.results[b * HPC]["y"].astype(np.float32)
        for c in range(b * HPC + 1, (b + 1) * HPC):
            acc = acc + res.results[c]["y"]
        out[b] = acc + bo
    return out, res


def kernel(**inputs) -> np.ndarray:
    out, _ = run(inputs, trace=False)
    return out


if __name__ == "__main__":
    rng = np.random.default_rng(0)
    ins = {
        "x": rng.standard_normal((B, N, D)).astype(np.float32),
        "wq": (rng.standard_normal((D, D)) * 0.02).astype(np.float32),
        "bq": (rng.standard_normal(D) * 0.02).astype(np.float32),
        "wk": (rng.standard_normal((D, D)) * 0.02).astype(np.float32),
        "bk": (rng.standard_normal(D) * 0.02).astype(np.float32),
        "wv": (rng.standard_normal((D, D)) * 0.02).astype(np.float32),
        "bv": (rng.standard_normal(D) * 0.02).astype(np.float32),
        "wo": (rng.standard_normal((D, D)) * 0.02).astype(np.float32),
        "bo": (rng.standard_normal(D) * 0.02).astype(np.float32),
    }
    print(kernel(**ins).shape)


# revision 20
# speedup vs baseline: 1.0049x; 1.0049x over previous
"""L2-distance self-attention (B=2, N=2048, D=1024, H=16) on 8 trn2 NeuronCores, v3.

Sharding: core c handles batch c//4 and heads 4*(c%4) .. 4*(c%4)+4.

Key HW facts driving the design (measured on this device):
  - PE matmuls stream 1 col/cycle at 2.4 GHz only when K > 64; K<=64 runs at
    half rate.  So all big contractions are padded to K=128 (dead rows are
    free - cost is F-column driven).
  - Matmul PSUM dst must stay within one 2KB bank -> all matmuls emit F=512.
  - fp8 DoubleRow (K=2x128) doubles matmul throughput; used for PV.
  - ACT runs ~0.9ns/col; DVE tensor_scalar ~0.42ns/col (fp16 2x mode).

Engine split:
  PE : fp16 projections (K=128 tiles), fp16 S^T (K=128-padded aug tiles),
       fp8e4 DoubleRow PV, 1/den broadcast matmul, fp16 out-proj.
  ACT: sqrt (u = A*s via scale=A^2), half the proj->aug copies (Identity with
       per-partition bias column), softmax reciprocal (ACT table, from PSUM).
  DVE: Schraudolph exp: bits16 = sat_round(B8 - u/128); LOW byte is exactly
       float8_e4m3 of exp(-s)*2^(7*log2e) (scale cancels in softmax); PV reads
       the bits buffer via a stride-2 fp8e4 view.  Plus copies + normalize.
"""

import sys

for p in ("/opt/trn_rl_repo", "/root/.axon_site/_ro/trn_rl_repo"):
    if p not in sys.path:
        sys.path.append(p)

import numpy as np

B, N, D, H = 2, 2048, 1024, 16
HD = 64          # head dim
HPC = 4          # heads per core
HS = HPC * HD    # head-group width per core (256)
NB = N // 128    # 16 j blocks
KB = D // 128    # 8 contraction blocks for projections

A_CONST = 1024.0 * np.log2(np.e)          # 1477.3197
A2_CONST = float(A_CONST * A_CONST)
SCHR_SCALE = -1.0 / 128.0
SCHR_B = float(8 * np.log2(np.e) * 7.0 + 56.0 - 8.0 * 0.043)   # 136.447

_CACHE = {}


def _enable_ldw_opt():
    """Let walrus dedupe back-to-back same-weight LDWEIGHTS (S^T emits 4
    matmuls per j-block sharing one lhsT).  bass hardcodes the flag off."""
    import concourse.bass_utils as _bu

    if getattr(_bu, "_ldw_opt_patched", False):
        return
    _orig = _bu.run_command

    def _patched(argv, **kwargs):
        argv = [
            "--enable-ldw-opt=true" if a == "--enable-ldw-opt=false" else a
            for a in argv
        ]
        return _orig(argv, **kwargs)

    _bu.run_command = _patched
    _bu._ldw_opt_patched = True


def _build():
    import concourse.bacc as bacc
    import concourse.mybir as mybir
    import concourse.tile as tile

    dt = mybir.dt
    AF = mybir.ActivationFunctionType
    ALU = mybir.AluOpType
    PM = mybir.MatmulPerfMode

    nc = bacc.Bacc("TRN2", target_bir_lowering=False, debug=False)

    # ---- DRAM I/O (per core) ----
    xT = nc.dram_tensor("xT", [D, N], dt.float16, kind="ExternalInput")
    wq = nc.dram_tensor("wqT", [D, HS], dt.float16, kind="ExternalInput")
    wk = nc.dram_tensor("wkT", [D, HS], dt.float16, kind="ExternalInput")
    wv = nc.dram_tensor("wv_aug", [D + 1, HS], dt.float16, kind="ExternalInput")
    wo = nc.dram_tensor("woT", [HS, D], dt.float16, kind="ExternalInput")
    bcols = nc.dram_tensor("bcols", [128, 4], dt.float32, kind="ExternalInput")
    y = nc.dram_tensor("y", [N, D], dt.float32, kind="ExternalOutput")

    def act_recip(out, in_):
        """ACT-table reciprocal (bass blocks the wrapper; emit directly)."""
        se = nc.scalar
        inputs = [se.lower_ap(in_)]
        for arg in (0.0, 1.0, 0.0):   # bias, scale, alpha
            inputs.append(mybir.ImmediateValue(dtype=mybir.dt.float32, value=arg))
        return se.add_instruction(
            mybir.InstActivation(
                name=se.bass.get_next_instruction_name(),
                func=AF.Reciprocal,
                ins=inputs,
                outs=[se.lower_ap(out)],
            )
        )

    with tile.TileContext(nc) as tc:
        with (
            tc.tile_pool(name="cst", bufs=1) as cst,
            tc.tile_pool(name="u4", bufs=2) as u4,
            tc.tile_pool(name="up", bufs=4) as up,
            tc.tile_pool(name="wp", bufs=1) as wp,
            tc.tile_pool(name="wop", bufs=1) as wop,
            tc.tile_pool(name="aug", bufs=1) as aug,
            tc.tile_pool(name="recp", bufs=2) as recp,
            tc.tile_pool(name="pss", bufs=2, space="PSUM") as pss,   # 2-bank slots
            tc.tile_pool(name="psv", bufs=1, space="PSUM") as psv,   # 2x 2-bank halves
        ):
            # ---- constants ----
            ones_row = cst.tile([1, N], dt.float16, tag="ones_row")
            nc.gpsimd.memset(ones_row[:], 1.0)
            # K=128-padded colsum producers: rows 0..63 active, 64..127 zero.
            # e66q col64 = 1 (q2 row); e66k col65 = 0.25 (k2 = 0.25*sum(kb2^2)).
            e66q = cst.tile([128, 66], dt.float16, tag="e66q")
            nc.gpsimd.memset(e66q[:], 0.0)
            nc.gpsimd.memset(e66q[0:64, 64:65], 1.0)
            e66k = cst.tile([128, 66], dt.float16, tag="e66k")
            nc.gpsimd.memset(e66k[:], 0.0)
            nc.gpsimd.memset(e66k[0:64, 65:66], 0.25)
            bcol = cst.tile([128, 4], dt.float32, tag="bcol")
            nc.sync.dma_start(bcol[:], bcols[:, :])
            ones_f32 = cst.tile([1, 2], dt.float32, tag="ones_f32")
            nc.gpsimd.memset(ones_f32[:], 1.0)
            # aug-row add-constants: col 0 = 1@row65 (q: ones row), col 1 =
            # 1@row64 (k: ones row); zero elsewhere.  Partition-65 writes are
            # only legal via DMA.
            rowc = cst.tile([128, 2], dt.float32, tag="rowc")
            nc.gpsimd.memset(rowc[:], 0.0)
            nc.sync.dma_start(rowc[65:66, 0:1], ones_f32[0:1, 0:1])
            nc.sync.dma_start(rowc[64:65, 1:2], ones_f32[0:1, 1:2])

            # PE warmup
            for w in range(2):
                wup = pss.tile([128, 512], dt.float32, tag="b2", name="wup")
                for r in range(12):
                    nc.tensor.matmul(
                        wup[:], ones_row[0:1, 0:128], ones_row[0:1, 0:512],
                        start=(r == 0), stop=(r == 11),
                    )

            # ---- load inputs ----
            xt = [u4.tile([128, N], dt.float16, tag="xt", name=f"xt{k}", bufs=8)
                  for k in range(KB)]
            wq_all = wp.tile([128, KB * HS], dt.float16, tag="wq_all")
            wk_all = wp.tile([128, KB * HS], dt.float16, tag="wk_all")
            wv_all = wp.tile([128, KB * HS], dt.float16, tag="wv_all")
            brows = wp.tile([1, HS], dt.float16, tag="brows")
            for k in range(KB):
                eng = nc.sync if k % 2 == 0 else nc.gpsimd
                eng.dma_start(xt[k][:], xT[k * 128 : (k + 1) * 128, :])
                nc.sync.dma_start(
                    wq_all[:, k * HS : (k + 1) * HS], wq[k * 128 : (k + 1) * 128, :]
                )
            for k in range(KB):
                nc.sync.dma_start(
                    wk_all[:, k * HS : (k + 1) * HS], wk[k * 128 : (k + 1) * 128, :]
                )
            nc.gpsimd.dma_start(brows[:, :], wv[D : D + 1, :])
            for k in range(KB):
                nc.gpsimd.dma_start(
                    wv_all[:, k * HS : (k + 1) * HS], wv[k * 128 : (k + 1) * 128, :]
                )

            # ---- big buffers (aug padded to K=128; rows 66..127 zero) ----
            q_aug = [aug.tile([128, N], dt.float16, tag=f"qa{h}", name=f"qa{h}")
                     for h in range(HPC)]
            k_stat = [aug.tile([128, N], dt.float16, tag=f"ks{h}", name=f"ks{h}")
                      for h in range(HPC)]
            for h in range(HPC):
                nc.gpsimd.memset(q_aug[h][:], 0.0)
                nc.gpsimd.memset(k_stat[h][:], 0.0)
            # v8: [128 j, (4 h, 16 jb, 128 m)] e4m3; m = [v(64) | ones | pad0]
            v8 = aug.tile([128, HPC * NB * 128], dt.float8e4, tag="v8")
            v8r = v8[:].rearrange("p (h t m) -> p h t m", h=HPC, m=128)
            nc.gpsimd.memset(v8[:], 0.0)
            nc.gpsimd.memset(v8r[:, :, :, 64:65], 1.0)
            # bits ring: 4 slots x [2 jb, 2048 i] u16
            bits = aug.tile([128, 4 * 2 * N], dt.uint16, tag="bits")
            bits_e4 = bits[:].bitcast(dt.float8e4)
            bits_r = bits[:].rearrange("p (s t i) -> p s t i", s=4, t=2)
            oTp = [
                aug.tile([128, N], dt.float16, tag="oTp0", name="oTp0"),
                aug.tile([128, N], dt.float16, tag="oTp1", name="oTp1"),
            ]
            oTraw = [
                aug.tile([128, N], dt.float16, tag="oTr0", name="oTr0"),
                aug.tile([128, N], dt.float16, tag="oTr1", name="oTr1"),
            ]
            den_sb = [aug.tile([1, N], dt.float16, tag=f"den{h}", name=f"den{h}")
                      for h in range(HPC)]

            # ---- q/k projections; bias added at the psum->aug copy ----
            def proj_qk_c(w_all, bc_idx, dest, m, c):
                p = pss.tile([128, 1024], dt.float32, tag="b2", name="pp")
                lo = c * 1024
                for sc in range(2):
                    fo = sc * 512
                    for k in range(KB):
                        nc.tensor.matmul(
                            p[:, fo : fo + 512],
                            w_all[:, k * HS + m * 128 : k * HS + (m + 1) * 128],
                            xt[k][:, lo + fo : lo + fo + 512],
                            start=(k == 0), stop=(k == KB - 1),
                        )
                # psum -> aug rows with per-partition bias column (DVE)
                nc.vector.tensor_scalar(
                    out=dest[2 * m][0:64, lo : lo + 1024],
                    in0=p[0:64, :],
                    scalar1=bcol[0:64, bc_idx : bc_idx + 1], scalar2=None,
                    op0=ALU.add,
                )
                nc.vector.tensor_scalar(
                    out=dest[2 * m + 1][0:64, lo : lo + 1024],
                    in0=p[64:128, :],
                    scalar1=bcol[64:128, bc_idx : bc_idx + 1], scalar2=None,
                    op0=ALU.add,
                )

            def norms_part(h, which):
                src_tile = q_aug[h] if which == "q" else k_stat[h]
                emat = e66q if which == "q" else e66k
                rc = 0 if which == "q" else 1
                sq = u4.tile([128, N], dt.float16, tag="sq", name="sq", bufs=2)
                nc.vector.tensor_tensor(
                    out=sq[:], in0=src_tile[:], in1=src_tile[:], op=ALU.mult,
                )
                for c in range(2):
                    lo = c * 1024
                    p = pss.tile([66, 1024], dt.float32, tag="b2", name="np")
                    for sc in range(2):
                        fo = sc * 512
                        nc.tensor.matmul(
                            p[:, fo : fo + 512], emat[:],
                            sq[:, lo + fo : lo + fo + 512],
                            start=True, stop=True,
                        )
                    # rows 64:66 = (q2, 0)+(0, 1) for q; (0, k2)+(1, 0) for k
                    nc.vector.tensor_scalar(
                        out=src_tile[64:66, lo : lo + 1024],
                        in0=p[64:66, :],
                        scalar1=rowc[64:66, rc : rc + 1], scalar2=None,
                        op0=ALU.add,
                    )

            # ---- v projection (bias added at the fp8 copy via bv bcast) ----
            bvb = wp.tile([128, HS], dt.float16, tag="bvb")

            def bvb_init():
                p = pss.tile([128, HS], dt.float32, tag="b2", name="bvbp")
                nc.tensor.matmul(p[:], ones_row[0:1, 0:128], brows[0:1, :],
                                 start=True, stop=True)
                nc.scalar.copy(bvb[:], p[:])

            def vp_one(jb):
                p = pss.tile([128, HS], dt.float32, tag="b2", name="vp")
                for k in range(KB):
                    nc.tensor.matmul(
                        p[:], xt[k][:, jb * 128 : (jb + 1) * 128],
                        wv_all[:, k * HS : (k + 1) * HS],
                        start=(k == 0), stop=(k == KB - 1),
                    )
                with nc.allow_low_precision(reason="fp8 attention values"):
                    nc.vector.tensor_tensor(
                        out=v8r[:, :, jb, 0:64],
                        in0=p[:].rearrange("p (h d) -> p h d", d=64),
                        in1=bvb[:].rearrange("p (h d) -> p h d", d=64),
                        op=ALU.add,
                    )

            # ---- S^T + sqrt + Schraudolph-exp ----
            def st_chunk(h, jb, c):
                lo = c * 1024
                stp = pss.tile([128, 1024], dt.float32, tag="b2", name="st")
                for sc in range(2):
                    fo = sc * 512
                    nc.tensor.matmul(
                        stp[:, fo : fo + 512],
                        k_stat[h][:, jb * 128 : (jb + 1) * 128],
                        q_aug[h][:, lo + fo : lo + fo + 512],
                        start=True, stop=True,
                    )
                u = up.tile([128, 1024], dt.float16, tag="u", name="u")
                nc.scalar.activation(u[:], stp[:], AF.Sqrt, scale=A2_CONST)
                slot = (jb // 2) % 4
                with nc.allow_low_precision(reason="fp8 softmax weights"):
                    nc.vector.tensor_scalar(
                        out=bits_r[:, slot, jb % 2, lo : lo + 1024],
                        in0=u[:], scalar1=SCHR_SCALE, scalar2=SCHR_B,
                        op0=ALU.mult, op1=ALU.add,
                    )

            def pv_pair(h, t, pv):
                slot = t % 4
                br = bits_e4.rearrange("p (s t i) -> p s t i", s=4, t=2)
                for sc in range(4):
                    bo_ = 2 * 512 * sc
                    half = pv[sc // 2]
                    nc.tensor.matmul(
                        half[:, 512 * (sc % 2) : 512 * (sc % 2) + 512],
                        v8r[:, h, 2 * t : 2 * t + 2, :],
                        br[:, slot, :, bo_ : bo_ + 1024 : 2],
                        start=(t == 0), stop=(t == NB // 2 - 1),
                        perf_mode=PM.DoubleRow,
                    )

            def normalize(h, pv):
                # Per-head: evacuate pv halves to SBUF only (reciprocal is
                # deferred to the tail: one ACT table switch total).  Each
                # 2-bank half releases independently so the next head's PV
                # can start while the other half is still draining.
                half = 64 * (h % 2)
                for c in range(2):
                    lo = c * 1024
                    with nc.allow_low_precision(reason="softmax denominator"):
                        nc.vector.tensor_copy(
                            den_sb[h][:, lo : lo + 1024], pv[c][64:65, :]
                        )
                        if h % 2 == 0:
                            nc.scalar.copy(
                                oTraw[h // 2][0:64, lo : lo + 1024], pv[c][0:64, :]
                            )
                        else:
                            nc.vector.tensor_copy(
                                oTraw[h // 2][64:128, lo : lo + 1024], pv[c][0:64, :]
                            )

            def normalize_tail(h):
                rec = recp.tile([1, N], dt.float16, tag="rec", name="rec")
                with nc.allow_low_precision(reason="softmax reciprocal"):
                    act_recip(rec[:], den_sb[h][:])
                half = 64 * (h % 2)
                for c in range(2):
                    lo = c * 1024
                    bc = pss.tile([128, 1024], dt.float32, tag="b2", name="bc")
                    for sc in range(2):
                        fo = sc * 512
                        nc.tensor.matmul(
                            bc[half : half + 64, fo : fo + 512],
                            ones_row[0:1, 0:64],
                            rec[0:1, lo + fo : lo + fo + 512],
                            start=True, stop=True,
                        )
                    bcs = u4.tile([128, 1024], dt.float16, tag="bcs", name="bcs",
                                  bufs=2)
                    nc.vector.tensor_copy(
                        bcs[half : half + 64, :], bc[half : half + 64, :]
                    )
                    with nc.allow_low_precision(reason="fp16 attention out"):
                        nc.vector.tensor_tensor(
                            out=oTp[h // 2][half : half + 64, lo : lo + 1024],
                            in0=oTraw[h // 2][half : half + 64, lo : lo + 1024],
                            in1=bcs[half : half + 64, :], op=ALU.mult,
                        )

            # ================= emission schedule =================
            proj_qk_c(wq_all, 0, q_aug, 0, 0)
            proj_qk_c(wk_all, 2, k_stat, 0, 0)
            proj_qk_c(wq_all, 0, q_aug, 0, 1)
            proj_qk_c(wk_all, 2, k_stat, 0, 1)
            norms_part(0, "q")
            norms_part(0, "k")
            norms_part(1, "q")
            norms_part(1, "k")

            wotp = [wop.tile([128, D], dt.float16, tag=f"wop{p}", name=f"wop{p}")
                    for p in range(2)]
            for p in range(2):
                nc.gpsimd.dma_start(wotp[p][:], wo[p * 128 : (p + 1) * 128, :])

            pv_tiles = {}

            def head_loop(h, weave):
                """weave[i] runs after jb i's st chunks, before the pair PV."""
                pv = [
                    psv.tile([128, 1024], dt.float32, tag=f"pv{c}", name=f"pv{h}_{c}")
                    for c in range(2)
                ]
                pv_tiles[h] = pv
                for jb in range(NB):
                    st_chunk(h, jb, 0)
                    st_chunk(h, jb, 1)
                    w = weave.get(jb)
                    if w:
                        w()
                    if jb % 2 == 1:
                        pv_pair(h, jb // 2, pv)

            bvb_init()
            vp_one(0)
            vp_one(1)
            weave0 = {}
            for jb in range(14):
                weave0[jb] = (lambda j=jb: vp_one(j + 2))
            weave0[6] = lambda: (vp_one(8), proj_qk_c(wq_all, 1, q_aug, 1, 0))
            weave0[7] = lambda: (vp_one(9), proj_qk_c(wq_all, 1, q_aug, 1, 1))
            weave0[8] = lambda: (vp_one(10), proj_qk_c(wk_all, 3, k_stat, 1, 0))
            weave0[9] = lambda: (vp_one(11), proj_qk_c(wk_all, 3, k_stat, 1, 1))
            weave0[10] = lambda: (vp_one(12), norms_part(2, "q"))
            weave0[11] = lambda: (vp_one(13), norms_part(2, "k"))
            weave0[12] = lambda: (vp_one(14), norms_part(3, "q"))
            weave0[13] = lambda: (vp_one(15), norms_part(3, "k"))
            head_loop(0, weave0)

            head_loop(1, {0: lambda: normalize(0, pv_tiles[0])})
            head_loop(2, {0: lambda: normalize(1, pv_tiles[1])})
            head_loop(3, {0: lambda: normalize(2, pv_tiles[2])})
            normalize(3, pv_tiles[3])
            for h in range(HPC):
                normalize_tail(h)

            # ---- output projection ----
            for ib in range(NB):
                yp = pss.tile([128, D], dt.float32, tag="b2", name="yp")
                for pr in range(2):
                    for sc in range(2):
                        fo = sc * 512
                        nc.tensor.matmul(
                            yp[:, fo : fo + 512],
                            oTp[pr][:, ib * 128 : (ib + 1) * 128],
                            wotp[pr][:, fo : fo + 512],
                            start=(pr == 0), stop=(pr == 1),
                        )
                yac = u4.tile([128, D], dt.float32, tag="yac", name="yac", bufs=2)
                if ib % 2 == 0:
                    nc.scalar.copy(yac[:], yp[:])
                    nc.sync.dma_start(y[ib * 128 : (ib + 1) * 128, :], yac[:])
                else:
                    nc.vector.tensor_copy(yac[:], yp[:])
                    nc.gpsimd.dma_start(y[ib * 128 : (ib + 1) * 128, :], yac[:])

    nc.compile()
    return nc


def _prep_in_maps(x, wq, bq, wk, bk, wv, bv, wo):
    f16 = np.float16
    in_maps = []
    xTs = [np.ascontiguousarray(x[b].T).astype(f16) for b in range(B)]
    for c in range(8):
        b, hg = divmod(c, HPC)
        hs = hg * HS
        wqT = np.ascontiguousarray(wq[hs : hs + HS, :].T).astype(f16)
        wkT = np.ascontiguousarray(-2.0 * wk[hs : hs + HS, :].T).astype(f16)
        wv_aug = np.concatenate(
            [wv[hs : hs + HS, :].T, bv[None, hs : hs + HS]], axis=0
        ).astype(f16)
        woT = np.ascontiguousarray(wo[:, hs : hs + HS].T).astype(f16)
        # bias columns [128, 4]: rows 0:64 even head, 64:128 odd head of the
        # m-pair; cols = (q m0, q m1, k m0, k m1); k pre-scaled by -2.
        bc = np.zeros((128, 4), np.float32)
        bc[:, 0] = bq[hs : hs + 128]
        bc[:, 1] = bq[hs + 128 : hs + 256]
        bc[:, 2] = -2.0 * bk[hs : hs + 128]
        bc[:, 3] = -2.0 * bk[hs + 128 : hs + 256]
        in_maps.append(
            {
                "xT": xTs[b],
                "wqT": wqT,
                "wkT": wkT,
                "wv_aug": np.ascontiguousarray(wv_aug),
                "woT": woT,
                "bcols": bc,
            }
        )
    return in_maps


def _get_nc():
    if "nc" not in _CACHE:
        _CACHE["nc"] = _build()
    return _CACHE["nc"]


def run(inputs, trace=False, **trace_kwargs):
    """Run on 8 cores; returns (full_output, BassKernelResults)."""
    from concourse.bass_utils import run_bass_kernel_spmd

    nc = _get_nc()
    in_maps = _prep_in_maps(
        np.asarray(inputs["x"], np.float32),
        np.asarray(inputs["wq"], np.float32), np.asarray(inputs["bq"], np.float32),
        np.asarray(inputs["wk"], np.float32), np.asarray(inputs["bk"], np.float32),
        np.asarray(inputs["wv"], np.float32), np.asarray(inputs["bv"], np.float32),
        np.asarray(inputs["wo"], np.float32),
    )
    res = run_bass_kernel_spmd(nc, in_maps, list(range(8)), trace=trace, **trace_kwargs)
    bo = np.asarray(inputs["bo"], np.float32)
    out = np.empty((B, N, D), np.float32)
    for b in range(B):
        acc = res.results[b * HPC]["y"].astype(np.float32)
        for c in range(b * HPC + 1, (b + 1) * HPC):
            acc = acc + res.results[c]["y"]
        out[b] = acc + bo
    return out, res


def kernel(**inputs) -> np.ndarray:
    out, _ = run(inputs, trace=False)
    return out


if __name__ == "__main__":
    rng = np.random.default_rng(0)
    ins = {
        "x": rng.standard_normal((B, N, D)).astype(np.float32),
        "wq": (rng.standard_normal((D, D)) * 0.02).astype(np.float32),
        "bq": (rng.standard_normal(D) * 0.02).astype(np.float32),
        "wk": (rng.standard_normal((D, D)) * 0.02).astype(np.float32),
        "bk": (rng.standard_normal(D) * 0.02).astype(np.float32),
        "wv": (rng.standard_normal((D, D)) * 0.02).astype(np.float32),
        "bv": (rng.standard_normal(D) * 0.02).astype(np.float32),
        "wo": (rng.standard_normal((D, D)) * 0.02).astype(np.float32),
        "bo": (rng.standard_normal(D) * 0.02).astype(np.float32),
    }
    print(kernel(**ins).shape)
